# revision 37
# baseline (speedup 1.0000x reference)
"""MoE SwiGLU experts kernel for Trainium2 (8 NeuronCores, expert-parallel).

Each core owns one expert e. Host does the dispatch (gathers tokens whose
top-k includes e, dedups with summed combine weights), transposes operands
so every matmul contraction dim lands on SBUF partitions, and pads the
token batch to capacity C. Device computes the SwiGLU MLP for its expert:

    gateT = gate_w[e] @ x_eT          # [I, C]  (contract H)
    upT   = up_w[e]   @ x_eT          # [I, C]
    interT = silu(gateT) * upT        # [I, C]
    outT  = down_w[e]  @ interT       # [H, C]  (contract I)

Host scatter-adds w * outT.T rows into the [T, H] output.
Matmuls run in bf16 with fp32 PSUM accumulation.

Hardware sync-wait slots per instruction are scarce (walrus rejects
kernels that need too many), so the structure keeps every instruction's
dependency fan-in tiny:
- all three weight tensors and the token slab are fully SBUF-resident and
  written once, so their DMAs carry no WAR/WAW waits;
- DVE tensor ops depend on exactly one semaphore (both multiplicands are
  produced by ACT; the destination is written once);
- slot-rotating buffers are only ever rewritten by ACT, which has enough
  wait slots for {producer, WAR, own} sets;
- output stores are batched into 8 DMAs so each lands on a fresh HW lane.
"""

import numpy as np


def _build_bass(C: int, H: int, I: int):
    from contextlib import ExitStack

    import concourse.bass as bass
    import concourse.mybir as mybir
    import concourse.tile as tile

    f32 = mybir.dt.float32
    bf16 = mybir.dt.bfloat16
    P = 128
    KH = H // P  # 16
    KI = I // P  # 11

    chunks = []
    off = 0
    while off < C:
        w = min(512, C - off)
        chunks.append((off, w))
        off += w

    nc = bass.Bass(dynamic_dma_scratch_size=8192)
    xT_d = nc.dram_tensor("xT", [H, C], bf16, kind="ExternalInput")
    gwT_d = nc.dram_tensor("gwT", [H, I], bf16, kind="ExternalInput")
    uwT_d = nc.dram_tensor("uwT", [H, I], bf16, kind="ExternalInput")
    dwT_d = nc.dram_tensor("dwT", [I, H], bf16, kind="ExternalInput")
    outT_d = nc.dram_tensor("outT", [H, C], bf16, kind="ExternalOutput")

    x3 = xT_d[:].rearrange("(o p) c -> p o c", p=P)
    gw3 = gwT_d[:].rearrange("(o p) i -> p o i", p=P)
    uw3 = uwT_d[:].rearrange("(o p) i -> p o i", p=P)
    dw3 = dwT_d[:].rearrange("(o p) h -> p o h", p=P)
    o3 = outT_d[:].rearrange("(o p) c -> p o c", p=P)

    with ExitStack() as ctx:
        tc = ctx.enter_context(tile.TileContext(nc))
        wpool = ctx.enter_context(tc.tile_pool(name="w", bufs=1))
        xpool = ctx.enter_context(tc.tile_pool(name="x", bufs=1))
        ipool = ctx.enter_context(tc.tile_pool(name="inter", bufs=1))
        opool = ctx.enter_context(tc.tile_pool(name="out", bufs=1))
        ppool = ctx.enter_context(tc.tile_pool(name="psum", bufs=1, space="PSUM"))
        ppool2 = ctx.enter_context(tc.tile_pool(name="psum2", bufs=2, space="PSUM"))

        # weights resident: every load writes a fresh region
        gw_sb = wpool.tile([P, KH, I], bf16, name="gw_sb")
        uw_sb = wpool.tile([P, KH, I], bf16, name="uw_sb")
        dw_sb = wpool.tile([P, KI, H], bf16, name="dw_sb")
        sw_dmas = []
        for kh in range(KH):
            nc.gpsimd.dma_start(gw_sb[:, kh], gw3[:, kh])
            nc.gpsimd.dma_start(uw_sb[:, kh], uw3[:, kh])
        for ki in range(KI):
            sw_dmas.append(nc.gpsimd.dma_start(dw_sb[:, ki], dw3[:, ki]))
        # x and out share one hand-allocated slab (x is dead after phase 1;
        # both hazard directions resolve to the PE semaphore)
        xo_arena = nc.alloc_sbuf_tensor("xo_arena", [P, KH * C], bf16)
        xo_off = nc.lookup_mloc(xo_arena).addr
        x_sb = nc.alloc_sbuf_tensor_at("x_sb", [P, KH, C], bf16, offset=xo_off)[:]
        out_sb = nc.alloc_sbuf_tensor_at(
            "out_sb", [P, KH, C], bf16, offset=xo_off
        )[:]
        x_dma = nc.sync.dma_start(x_sb[:], x3)
        inter_sb = ipool.tile([P, KI, C], bf16, name="inter_sb")
        puc_sb = ipool.tile([P, KI, C], bf16, name="puc_sb")
        fence_t = ipool.tile([P, 16], bf16, name="fence_t")
        last_tt = [None]

        # ---- phase 1: interT = silu(gateT) * upT ----
        for im in range(KI):
            pg = [
                ppool.tile([P, w], f32, tag=f"a{j}", name=f"pg{j}")
                for j, (_, w) in enumerate(chunks)
            ]
            pu = [
                ppool.tile([P, w], f32, tag=f"b{j}", name=f"pu{j}")
                for j, (_, w) in enumerate(chunks)
            ]
            for kh in range(KH):
                for j, (o, w) in enumerate(chunks):
                    nc.tensor.matmul(
                        pg[j][:],
                        gw_sb[:, kh, im * P : (im + 1) * P],
                        x_sb[:, kh, o : o + w],
                        start=(kh == 0),
                        stop=(kh == KH - 1),
                    )
            for kh in range(KH):
                for j, (o, w) in enumerate(chunks):
                    nc.tensor.matmul(
                        pu[j][:],
                        uw_sb[:, kh, im * P : (im + 1) * P],
                        x_sb[:, kh, o : o + w],
                        start=(kh == 0),
                        stop=(kh == KH - 1),
                    )
            for j, (o, w) in enumerate(chunks):
                # ACT evacuates both PSUM tiles into write-once resident
                # SBUF buffers ({PE} is each copy's only wait); the DVE
                # multiply then reads two ACT products - one semaphore
                nc.scalar.activation(
                    inter_sb[:, im, o : o + w],
                    pg[j][:],
                    mybir.ActivationFunctionType.Silu,
                )
                nc.scalar.copy(puc_sb[:, im, o : o + w], pu[j][:])
                last_tt[0] = nc.vector.tensor_mul(
                    inter_sb[:, im, o : o + w],
                    inter_sb[:, im, o : o + w],
                    puc_sb[:, im, o : o + w],
                )

        # ---- phase 2: outT = down_w @ interT ----
        # absorb the x-DMA completion into the ACT proc via a pure sync
        # edge, so the out copies' WAW-vs-x fence is already observed and
        # each copy's only wait is the PE semaphore
        from concourse.tile import add_dep_helper

        fence = nc.scalar.copy(fence_t[:1, 0:8], fence_t[:1, 0:8])
        add_dep_helper(fence.ins, x_dma.ins, sync=True, reason="x lane fence")
        # sacrificial first reader of the aliased slab: takes the one-time
        # WAW-vs-x fence so the real output stores don't carry it
        scr_d = nc.dram_tensor("scr", [1, 16], bf16)
        dummy_store = nc.sync.dma_start(scr_d[:], x_sb[0:1, 0, 0:16])
        fence2 = nc.scalar.copy(fence_t[:1, 8:16], fence_t[:1, 8:16])
        add_dep_helper(fence2.ins, dummy_store.ins, sync=True, reason="slab fence")
        # absorb the last TT's DVE tick into the PE proc so phase-2 matmuls
        # wait only on their dw DMA lane
        pe_fence = nc.tensor.ldweights(gw_sb[:, 0, 0:1])
        add_dep_helper(pe_fence.ins, last_tt[0].ins, sync=True, reason="pe fence")
        hw_dmas = [x_dma, dummy_store]
        tail_insts = []
        last_mm = []
        for hm in range(KH):
            po = [
                ppool2.tile([P, w], f32, tag=f"c{j}", name=f"po{j}")
                for j, (_, w) in enumerate(chunks)
            ]
            for ki in range(KI):
                for j, (o, w) in enumerate(chunks):
                    last_mm.append(nc.tensor.matmul(
                        po[j][:],
                        dw_sb[:, ki, hm * P : (hm + 1) * P],
                        inter_sb[:, ki, o : o + w],
                        start=(ki == 0),
                        stop=(ki == KI - 1),
                    ))
                    del last_mm[:-1]
            for j, (o, w) in enumerate(chunks):
                tail_insts.append(
                    nc.scalar.copy(out_sb[:, hm, o : o + w], po[j][:])
                )
            # 7 stores (6x2 rows + final 4): with the dw load that is 8 HW
            # DMAs total - each lands on a fresh HW lane (no ring wait)
            if hm % 2 == 1 and hm < 10:
                hw_dmas.append(nc.sync.dma_start(
                    o3[:, hm - 1 : hm + 1, :], out_sb[:, hm - 1 : hm + 1]
                ))
            elif hm == 15:
                hw_dmas.append(nc.sync.dma_start(
                    o3[:, 10:16, :], out_sb[:, 10:16]
                ))

        # pre-drain: absorb every proc's final tick into the SP sequencer
        # one sync edge at a time, so the kernel-tail drain's waits (which
        # would exceed the instruction's wait slots) are all elided
        for insts in (sw_dmas, hw_dmas, [last_tt[0]], tail_insts[-2:], last_mm[-1:]):
            for bi in insts:
                if bi is None:
                    continue
                nop = nc.sync.nop()
                add_dep_helper(nop.ins, bi.ins, sync=True, reason="pre-drain")

    return nc


def kernel(hidden_states, top_k_index, top_k_weights, gate_w, up_w, down_w):
    import ml_dtypes
    from concourse.bass_utils import run_bass_kernel_spmd

    bf = ml_dtypes.bfloat16
    hs = np.ascontiguousarray(np.asarray(hidden_states, dtype=np.float32))
    tki = np.asarray(top_k_index)
    tkw = np.asarray(top_k_weights, dtype=np.float32)
    gw = np.asarray(gate_w, dtype=np.float32)
    uw = np.asarray(up_w, dtype=np.float32)
    dw = np.asarray(down_w, dtype=np.float32)

    T, H = hs.shape
    E, I, _ = gw.shape

    tok_lists, w_lists = [], []
    for e in range(E):
        mask = tki == e
        toks = np.nonzero(mask.any(axis=1))[0]
        w = (tkw * mask).sum(axis=1)[toks].astype(np.float32)
        tok_lists.append(toks)
        w_lists.append(w)

    # fixed capacity (PSUM/SBUF plan is sized for C=1024); experts with
    # more tokens (never happens for balanced routing at T=4096, K=2, E=8)
    # spill into additional SPMD rounds
    C = 1024
    n_rounds = max(1, -(-max(len(t) for t in tok_lists) // C))

    wT = [np.ascontiguousarray(gw[e].T.astype(bf)) for e in range(E)]
    uT = [np.ascontiguousarray(uw[e].T.astype(bf)) for e in range(E)]
    dT = [np.ascontiguousarray(dw[e].T.astype(bf)) for e in range(E)]

    nc = _build_bass(C, H, I)
    out = np.zeros((T, H), np.float32)
    global _last_results
    for r in range(n_rounds):
        in_maps = []
        for e in range(E):
            toks = tok_lists[e][r * C : (r + 1) * C]
            xT = np.zeros((H, C), bf)
            xT[:, : len(toks)] = hs[toks].T.astype(bf)
            in_maps.append(
                {"xT": xT, "gwT": wT[e], "uwT": uT[e], "dwT": dT[e]}
            )
        res = run_bass_kernel_spmd(nc, in_maps, core_ids=list(range(E)))
        _last_results = res
        for e in range(E):
            toks = tok_lists[e][r * C : (r + 1) * C]
            n = len(toks)
            if n == 0:
                continue
            outT_e = np.asarray(res.results[e]["outT"]).astype(np.float32)
            out[toks] += w_lists[e][r * C : r * C + n, None] * outT_e[:, :n].T
    return out
